# revision 1
# baseline (speedup 1.0000x reference)
"""Trainium2 Bass kernel for nn_PhaseLoss: three wrapped phase-loss terms.

loss = mean(unwrap(d)) + mean(unwrap(shift_diff_freq(d))) + mean(unwrap(shift_diff_time(d)))
with d = angle(ref) - angle(est), unwrap(x) = |x - 2pi*round(x/2pi)|.

Strategy (pure data parallel over batch, 8 cores, 4 batches each):
  - w = wrap(d) computed per element as the angle of z = ref * conj(est)
    via the half-angle identity:
      q = di / (hypot(dr, di) + |dr|)   in [-1, 1]   (sign of di rides q)
      t = arctan(q)                      in [-pi/4, pi/4]
      w = dr >= 0 ?  2t  :  sign(t)*pi - 2t          in [-pi, pi]
    (w == d mod 2pi, which is all the loss terms depend on)
  - partition dim = freq (f in [0,512), 4 tiles x 128), free dim = (batch, time);
    each freq tile is processed as BSPLIT batch-subunits for finer pipelining
  - products done in bf16 (ScalarE converts), angle pipeline via fused
    custom DVE ops, freq-shift via SBUF->SBUF DMA one partition down
  - per-partition partial sums go to a [128,48] output; host reduces
  - the f=512 leftover row, f=0 / t=0 shift_diff boundary terms are
    computed on host directly from the raw inputs (0.4% of elements)
"""

import numpy as np

B, F, T = 32, 513, 512
NCORES = 8
BPC = B // NCORES  # 4
NTILES = 4  # 4 x 128 partitions = f in [0, 512)
FREE = BPC * T  # 2048
PI = float(np.pi)
TWO_PI = 2.0 * PI
IN_NAMES = ("spec_est_real", "spec_est_imag", "spec_ref_real", "spec_ref_imag")

_CACHE: dict = {}
TRACE = False  # set by test harness to collect an NTFF profile
LAST_RESULT = None

# perf toggles (validated via cost-model timeline sim + HW)
OFFLOAD_DRDI = False  # dr/di adds on GpSimd instead of VectorE
OFFLOAD_Q = True  # q = di*inv on GpSimd
FUSED_RECIP = True  # single custom op: den = |dr|+hy, inv = ~1/den (1 Newton)
POOL_CONVERTS = 1  # how many of the 4 per-tile converts go to GpSimd
RAW_BUFS = 5  # input tile double-buffering depth
REPEAT = 1  # unroll the whole compute R times (benchmarking; output unchanged)
ZZ_ON_ACT = False  # zz = dr^2+di^2 via ScalarE squares instead of a DVE custom
# NOTE: BSPLIT=2 looked better in the cost-model sim (91.8us vs 97.1us cold)
# but measured ~147us/iter on HW vs ~93us for BSPLIT=1 - real per-instruction
# overheads punish the doubled instruction count. Keep coarse tiles.
BSPLIT = 1  # split each 4-batch tile into BSPLIT units
ROUNDS = 1  # table-set rounds: sqrt-batch then atan-batch per group of units
# MERGED_P2=True collapses phase 2 to 5 whole-width instructions; measured
# ~136us/iter on HW vs ~93us per-tile (serialization outweighs the saved
# instruction overheads). Keep per-tile phase 2.
MERGED_P2 = False


# --------------------------------------------------------------------------- #
# custom DVE ops
# --------------------------------------------------------------------------- #
def _get_ops():
    if "ops" in _CACHE:
        return _CACHE["ops"]
    import concourse.dve_ops as dve_ops
    from concourse.dve_ops import DveOp
    from concourse.dve_spec import (
        C0,
        C1,
        C2,
        Bin,
        Spec,
        Src0,
        Src1,
        Zero,
        _has_src1,
        lower,
        maxx,
        minn,
        select,
    )
    from concourse.dve_uop import AluOp, DveOpSpec
    from operator import add

    def mk(name, spec, subdim=False):
        for op in dve_ops.OPS:
            if op.name == name:
                return op
        shas = {}
        for ver in ("v3", "v4"):
            try:
                shas[ver] = DveOpSpec(
                    name=name, uops=lower(spec, ver=ver), rd1_en=_has_src1(spec)
                ).sha(ver)
            except Exception:
                pass
        op = DveOp(name, spec, subdim=subdim, uops_sha=shas)
        dve_ops.OPS.append(op)
        dve_ops._SUB_OPCODE_FOR_NAME[op.name] = (
            dve_ops._CUSTOM_DVE_ROW_BASE + len(dve_ops.OPS) - 1
        )
        dve_ops.CUSTOM_DVE_SPECS[op.name] = op.spec
        return op

    # zz = dr^2 + di^2
    sqsum = mk(
        "PL_SQSUM",
        Spec(
            body=Src0 * Src0 + Src1 * Src1,
            reference=lambda in0, in1, s0, s1, imm2: in0.astype(np.float32) ** 2
            + in1.astype(np.float32) ** 2,
        ),
    )
    # den = hy + |dr| + eps      (Src0 = hy, Src1 = dr, C0 = eps)
    absadd = mk(
        "PL_ABS_ADD",
        Spec(
            body=maxx(Src1, -Src1) + Src0 + C0,
            reference=lambda in0, in1, s0, s1, imm2: np.abs(in1).astype(np.float32)
            + in0
            + s0,
        ),
    )
    # w = select(dr<0, select(t<0,-pi,pi) - 2t, 2t)   (Src0 = t, Src1 = dr, C0 = pi)
    _h = Src0 + Src0
    _sp = select(Src0 < Zero, Zero - C0, C0)
    wfix = mk(
        "PL_WFIX",
        Spec(
            body=select(Src1 < Zero, _sp - _h, _h),
            reference=lambda in0, in1, s0, s1, imm2: np.where(
                in1 < 0,
                np.where(in0 < 0, -s0, s0).astype(np.float32) - 2 * in0,
                2 * in0,
            ).astype(np.float32),
        ),
    )
    # out = min(|a-b|, 2pi-|a-b|); accum_out = sum   (Src0=a, Src1=b, C0=2pi)
    # inv = ~1/(|dr| + hy + guard), one Newton step from the ~6% bitflip seed
    # (Src0 = hy, Src1 = dr; C0/C1 = seed consts, imm2 = zero guard)
    _absdr = Bin(AluOp.ABSOLUTE_VALUE, Src1, Src1)
    _den = maxx(_absdr + Src0, C2)
    _nx = Bin(AluOp.BITWISE_NOT, _den, _den)
    _y0 = _nx * C0
    _y1 = _y0 * (C1 - _den * _y0)

    def _dr_ref(in0, in1, s0, s1, imm2):
        den = np.maximum(np.abs(in1).astype(np.float32) + in0, imm2).astype(
            np.float32
        )
        nx = (~den.view(np.int32)).view(np.float32)
        y0 = nx * np.float32(s0)
        return (y0 * (np.float32(s1) - den * y0)).astype(np.float32)

    denrecip = mk(
        "PL_DEN_RECIP",
        Spec(body=_y1, reference=_dr_ref),
    )

    _d = Bin(AluOp.ABSOLUTE_DIFF, Src0, Src1)
    _a = _d

    def _wd_ref(in0, in1, s0, s1, imm2):
        ad = np.abs(in0.astype(np.float32) - in1.astype(np.float32))
        b = np.minimum(ad, s0 - ad).astype(np.float32)
        return b, b.reshape(b.shape[0], -1).sum(axis=-1, keepdims=True)

    wrapdiff = mk(
        "PL_WRAPDIFF",
        Spec(
            body=minn(_a, C0 - _a),
            accum=add,
            accum_init=Zero,
            reference=_wd_ref,
        ),
    )
    _CACHE["ops"] = (sqsum, absadd, wfix, wrapdiff, denrecip)
    return _CACHE["ops"]


# --------------------------------------------------------------------------- #
# bass program (identical on all 8 cores)
# --------------------------------------------------------------------------- #
def _build_bass():
    if "nc" in _CACHE:
        return _CACHE["nc"]
    import concourse.bacc as bacc
    import concourse.tile as tile
    from concourse import mybir

    sqsum, absadd, wfix, wrapdiff, denrecip = _get_ops()
    from concourse.dve_ops import RECIP_APPROX_FAST_CONSTS as RC

    dt = mybir.dt
    AF = mybir.ActivationFunctionType

    nc = bacc.Bacc("TRN2", name="phase_loss")
    ins = {
        n: nc.dram_tensor(n, [BPC, F, T], dt.float32, kind="ExternalInput")
        for n in IN_NAMES
    }
    out_d = nc.dram_tensor("partials", [128, 48], dt.float32, kind="ExternalOutput")

    with tile.TileContext(nc) as tc:
        with (
            tc.tile_pool(name="raw", bufs=RAW_BUFS) as p_raw,
            tc.tile_pool(name="uv", bufs=1) as p_uv,
            tc.tile_pool(name="prod", bufs=1) as p_prod,
            tc.tile_pool(name="pers", bufs=1 if MERGED_P2 else 4 * BSPLIT) as p_pers,
            tc.tile_pool(name="scr", bufs=2) as p_scr,
            tc.tile_pool(name="tsc", bufs=1 if MERGED_P2 else 4) as p_t,
            tc.tile_pool(name="wp", bufs=BSPLIT + 1) as p_w,
            tc.tile_pool(name="junk", bufs=2) as p_junk,
            tc.tile_pool(name="accp", bufs=1) as p_acc,
        ):
            acc = p_acc.tile([128, 48], dt.float32, tag="acc")
            nc.vector.memset(acc[:], 0.0)

            for _rep in range(REPEAT):
                ub = BPC // BSPLIT  # batches per unit
                FR = ub * T  # free elements per unit
                units = [(k, h) for k in range(NTILES) for h in range(BSPLIT)]
                NU = len(units)
                dr_tiles = {}
                q_tiles = {}
                w_prev = {}
                from concourse.tile_rust import add_dep_helper

                if MERGED_P2:
                    # contiguous across units so phase 2 runs as whole-width ops
                    DR_ALL = p_pers.tile([128, NU * FR], dt.bfloat16, tag="drall")
                    Q_ALL = p_pers.tile([128, NU * FR], dt.bfloat16, tag="qall")

                def phase1(ui, k, h, sqrt_insts, prev_atan):
                    raws = []
                    for n in IN_NAMES:
                        r = p_raw.tile([128, ub, T], dt.float32, tag="raw")
                        src = ins[n][
                            h * ub : (h + 1) * ub, 128 * k : 128 * (k + 1), :
                        ].rearrange("b f t -> f b t")
                        nc.sync.dma_start(r[:], src)
                        raws.append(r)
                    gr, gi, rr, ri = raws  # est_re, est_im, ref_re, ref_im
                    U = p_uv.tile([128, 2 * FR], dt.bfloat16, tag="U")
                    V = p_uv.tile([128, 2 * FR], dt.bfloat16, tag="V")
                    # U = (rr | ri), V = (gr | gi)
                    conv = [
                        (U[:, 0:FR], rr),
                        (U[:, FR:], ri),
                        (V[:, 0:FR], gr),
                        (V[:, FR:], gi),
                    ]
                    for ci, (dst, srct) in enumerate(conv):
                        srcv = srct[:].rearrange("p b t -> p (b t)")
                        if ci < POOL_CONVERTS:
                            nc.gpsimd.tensor_copy(dst, srcv)
                        else:
                            nc.scalar.copy(dst, srcv)

                    P = p_prod.tile([128, 4 * FR], dt.bfloat16, tag="P")
                    # (p1|p2) = (rr*gr | ri*gi) element-aligned -> one wide TT
                    nc.vector.tensor_mul(
                        P[:, 0 : 2 * FR], U[:, 0 : 2 * FR], V[:, 0 : 2 * FR]
                    )
                    nc.vector.tensor_mul(P[:, 2 * FR : 3 * FR], U[:, FR:], V[:, :FR])
                    nc.vector.tensor_mul(P[:, 3 * FR : 4 * FR], U[:, :FR], V[:, FR:])

                    if MERGED_P2:
                        drk = DR_ALL[:, ui * FR : (ui + 1) * FR]
                    else:
                        drt = p_pers.tile([128, FR], dt.bfloat16, tag="dr")
                        drk = drt[:]
                    dik = p_scr.tile([128, FR], dt.bfloat16, tag="di")
                    # dr = rr*gr + ri*gi ; di = ri*gr - rr*gi
                    eng_dd = nc.gpsimd if OFFLOAD_DRDI else nc.vector
                    eng_dd.tensor_add(drk, P[:, 0:FR], P[:, FR : 2 * FR])
                    eng_dd.tensor_sub(dik[:], P[:, 2 * FR : 3 * FR], P[:, 3 * FR :])

                    zz = p_scr.tile([128, FR], dt.float32, tag="s32")
                    nc.vector._custom_dve(sqsum, out=zz[:], in0=drk, in1=dik[:])
                    hy = p_scr.tile([128, FR], dt.bfloat16, tag="hy")
                    sq = nc.scalar.activation(hy[:], zz[:], AF.Sqrt)
                    if prev_atan is not None:
                        # keep ScalarE table sets batched per round
                        add_dep_helper(
                            sq.ins, prev_atan.ins, sync=True,
                            reason="table-set round ordering",
                        )
                    sqrt_insts.append(sq)
                    inv = p_scr.tile([128, FR], dt.float32, tag="s32")
                    if FUSED_RECIP:
                        nc.vector._custom_dve(
                            denrecip,
                            out=inv[:],
                            in0=hy[:],
                            in1=drk,
                            s0=RC["s0"],
                            s1=RC["s1"],
                            imm2=1e-30,
                        )
                    else:
                        den = p_scr.tile([128, FR], dt.float32, tag="s32")
                        nc.vector._custom_dve(
                            absadd, out=den[:], in0=hy[:], in1=drk, s0=1e-30
                        )
                        nc.vector.reciprocal_approx_fast(out=inv[:], in_=den[:])
                    if MERGED_P2:
                        qk = Q_ALL[:, ui * FR : (ui + 1) * FR]
                    else:
                        qt = p_pers.tile([128, FR], dt.bfloat16, tag="q")
                        qk = qt[:]
                    (nc.gpsimd if OFFLOAD_Q else nc.vector).tensor_mul(
                        qk, dik[:], inv[:]
                    )
                    dr_tiles[k, h] = drk
                    q_tiles[k, h] = qk

                def phase2(ui, k, h, last_sqrt):
                    tk = p_t.tile([128, FR], dt.bfloat16, tag="b16s")
                    atan_inst = nc.scalar.activation(
                        tk[:], q_tiles[k, h][:], AF.Arctan
                    )
                    add_dep_helper(
                        atan_inst.ins,
                        last_sqrt.ins,
                        sync=True,
                        reason="batch activation table sets",
                    )
                    wk = p_w.tile([128, FR], dt.bfloat16, tag="w")
                    nc.vector._custom_dve(
                        wfix, out=wk[:], in0=tk[:], in1=dr_tiles[k, h][:], s0=PI
                    )
                    # freq-shifted copy (one partition down) for the gd term
                    ws = p_w.tile([128, FR], dt.bfloat16, tag="ws")
                    nc.sync.dma_start(ws[1:128, :], wk[0:127, :])
                    if k > 0:
                        nc.sync.dma_start(ws[0:1, :], w_prev[h][127:128, :])
                    else:
                        # f=0 has no neighbor: ws[0]=w[0] zeroes that row's
                        # contribution; the f=0 term is done on host
                        nc.sync.dma_start(ws[0:1, :], wk[0:1, :])
                    # ip term: sum |w|
                    j1 = p_junk.tile([128, FR], dt.bfloat16, tag="junk")
                    nc.scalar.activation(
                        j1[:], wk[:], AF.Abs, accum_out=acc[:, ui : ui + 1]
                    )
                    # ptd term: time diffs within each batch row
                    w3 = wk[:].rearrange("p (b t) -> p b t", b=ub)
                    j2 = p_junk.tile([128, FR], dt.bfloat16, tag="junk")
                    j2v = j2[:].rearrange("p (b t) -> p b t", b=ub)
                    nc.vector._custom_dve(
                        wrapdiff,
                        out=j2v[:, :, 0 : T - 1],
                        in0=w3[:, :, 0 : T - 1],
                        in1=w3[:, :, 1:T],
                        s0=TWO_PI,
                        accum_out=acc[:, 16 + ui : 17 + ui],
                    )
                    # gd term: freq diffs
                    j3 = p_junk.tile([128, FR], dt.bfloat16, tag="junk")
                    nc.vector._custom_dve(
                        wrapdiff,
                        out=j3[:],
                        in0=ws[:],
                        in1=wk[:],
                        s0=TWO_PI,
                        accum_out=acc[:, 32 + ui : 33 + ui],
                    )
                    w_prev[h] = wk
                    return atan_inst

                if MERGED_P2:
                    sqrt_insts = []
                    for ui, (k, h) in enumerate(units):
                        phase1(ui, k, h, sqrt_insts, None)
                    # phase 2 as five whole-width instructions + 3 DMAs
                    W = NU * FR
                    T_ALL = p_t.tile([128, W], dt.bfloat16, tag="tall")
                    atan_inst = nc.scalar.activation(
                        T_ALL[:], Q_ALL[:], AF.Arctan
                    )
                    add_dep_helper(
                        atan_inst.ins,
                        sqrt_insts[-1].ins,
                        sync=True,
                        reason="batch activation table sets",
                    )
                    W_ALL = p_t.tile([128, W], dt.bfloat16, tag="wall")
                    nc.vector._custom_dve(
                        wfix, out=W_ALL[:], in0=T_ALL[:], in1=DR_ALL[:], s0=PI
                    )
                    WS_ALL = p_t.tile([128, W], dt.bfloat16, tag="wsall")
                    nc.sync.dma_start(WS_ALL[1:128, :], W_ALL[0:127, :])
                    # row 0 of each freq-block k>0 comes from block k-1's row 127
                    bs = BSPLIT * FR  # one freq-block's width
                    nc.sync.dma_start(
                        WS_ALL[0:1, bs:W], W_ALL[127:128, 0 : W - bs]
                    )
                    # f=0 blocks: ws=w makes that row's contribution zero
                    nc.sync.dma_start(WS_ALL[0:1, 0:bs], W_ALL[0:1, 0:bs])
                    # ip term: sum |w| (junk main output reuses the P slot)
                    j1 = p_prod.tile([128, W], dt.bfloat16, tag="P")
                    nc.scalar.activation(
                        j1[:], W_ALL[:], AF.Abs, accum_out=acc[:, 0:1]
                    )
                    # ptd term: time diffs within each (unit, batch) row
                    w3 = W_ALL[:].rearrange("p (s t) -> p s t", t=T)
                    t3 = T_ALL[:].rearrange("p (s t) -> p s t", t=T)
                    nc.vector._custom_dve(
                        wrapdiff,
                        out=t3[:, :, 0 : T - 1],
                        in0=w3[:, :, 0 : T - 1],
                        in1=w3[:, :, 1:T],
                        s0=TWO_PI,
                        accum_out=acc[:, 16:17],
                    )
                    # gd term: freq diffs (main output overwrites Q_ALL, dead)
                    nc.vector._custom_dve(
                        wrapdiff,
                        out=Q_ALL[:],
                        in0=WS_ALL[:],
                        in1=W_ALL[:],
                        s0=TWO_PI,
                        accum_out=acc[:, 32:33],
                    )
                else:
                    per = max(1, len(units) // ROUNDS)
                    groups = [
                        units[i : i + per] for i in range(0, len(units), per)
                    ]
                    prev_atan = None
                    for grp in groups:
                        sqrt_insts = []
                        for k, h in grp:
                            ui = units.index((k, h))
                            phase1(ui, k, h, sqrt_insts, prev_atan)
                        for k, h in grp:
                            ui = units.index((k, h))
                            prev_atan = phase2(ui, k, h, sqrt_insts[-1])
            nc.sync.dma_start(out_d[:], acc[:])

    nc.compile()
    _CACHE["nc"] = nc
    return nc


# --------------------------------------------------------------------------- #
# host-side boundary terms (f=512 row, f=0 gd term, t=0 ptd term)
# --------------------------------------------------------------------------- #
def _unwrap_np(x):
    return np.abs(x - TWO_PI * np.round(x / TWO_PI))


def _host_terms(gr, gi, rr, ri):
    """gr/gi/rr/ri: [B, F, T] float32 (est_real, est_imag, ref_real, ref_imag)."""

    def d_of(fsl):
        pr = np.arctan2(ri[:, fsl], rr[:, fsl]).astype(np.float64)
        pg = np.arctan2(gi[:, fsl], gr[:, fsl]).astype(np.float64)
        return pr - pg

    d_rows = d_of(slice(F - 2, F))  # [B, 2, T]: f = 511, 512
    d_f0 = d_of(0)  # [B, T]
    d_t0 = np.arctan2(ri[:, :, 0], rr[:, :, 0]).astype(np.float64) - np.arctan2(
        gi[:, :, 0], gr[:, :, 0]
    ).astype(np.float64)  # [B, F]

    h_ip = _unwrap_np(d_rows[:, 1]).sum()
    h_gd = _unwrap_np(d_f0).sum() + _unwrap_np(d_rows[:, 0] - d_rows[:, 1]).sum()
    h_ptd = (
        _unwrap_np(d_t0).sum()
        + _unwrap_np(d_rows[:, 1, :-1] - d_rows[:, 1, 1:]).sum()
    )
    return h_ip + h_gd + h_ptd


# --------------------------------------------------------------------------- #
# entry point
# --------------------------------------------------------------------------- #
def kernel(**inputs) -> np.ndarray:
    from concourse.bass_utils import run_bass_kernel_spmd

    full = {n: np.ascontiguousarray(np.asarray(inputs[n], dtype=np.float32)) for n in IN_NAMES}
    # drop the singleton channel dim -> [B, F, T]
    sq = {n: full[n].reshape(B, F, T) for n in IN_NAMES}

    nc = _build_bass()
    in_maps = [
        {n: np.ascontiguousarray(sq[n][c * BPC : (c + 1) * BPC]) for n in IN_NAMES}
        for c in range(NCORES)
    ]
    global LAST_RESULT
    for _attempt in range(3):
        res = run_bass_kernel_spmd(
            nc, in_maps, core_ids=list(range(NCORES)), trace=TRACE
        )
        LAST_RESULT = res
        parts = [r["partials"].astype(np.float64) for r in res.results]
        if all(np.isfinite(p).all() for p in parts):
            break
    dev_sum = float(sum(p.sum() for p in parts))

    host_sum = _host_terms(
        sq["spec_est_real"], sq["spec_est_imag"], sq["spec_ref_real"], sq["spec_ref_imag"]
    )
    n = float(B * F * T)
    return np.float32((dev_sum + host_sum) / n)

